# revision 1
# baseline (speedup 1.0000x reference)
"""Trainium2 Bass kernel for LocalGlobalSelfAttention.

Sharding: 8 cores = 4 batches x 2 sequence-halves (no collectives).
Each core computes, for its (batch b, half h):
  - global attention: queries = its half (SH rows), keys/values = full seq
  - local windowed attention: fully contained in its half
  - output projections (g+l accumulated in PSUM) + bias + residual + layernorm

Host side passes x^T with the core's own half FIRST (column-permuted; attention
is permutation-invariant over key positions) so the device can statically slice
queries as columns [0:SH]. Weights are host-converted to bf16. Matmuls run bf16
with fp32 PSUM accumulation. Softmax skips the max-subtraction (scores are
O(1) here) and exp() runs on ScalarE directly from PSUM; rowsums ride along the
AV matmul via a ones-column appended to V; normalization happens after AV.
"""

import numpy as np
import ml_dtypes
from collections import deque
from contextlib import ExitStack

BF16 = ml_dtypes.bfloat16

FULL_CFG = dict(S=2048, D=1024, H=16, K=64, NW=8)
N_CORES = 8
LN_EPS = 1e-3


def _chunks(total, size):
    return [(o, min(size, total - o)) for o in range(0, total, size)]


def build_nc(cfg=None):
    """Build + compile the per-core Bass program (SPMD, same on all cores)."""
    import concourse.bass as bass
    import concourse.tile as tile
    import concourse.mybir as mybir
    from concourse import bacc

    cfg = dict(cfg or FULL_CFG)
    S, D, H, K, NW = cfg["S"], cfg["D"], cfg["H"], cfg["K"], cfg["NW"]
    HK = H * K
    SH = S // 2          # per-core query rows (half the sequence)
    WIN = S // NW        # local attention window
    NWH = SH // WIN      # windows in this core's half
    assert K == 64 and D % 128 == 0 and HK % 128 == 0
    assert SH % 128 == 0 and WIN % 128 == 0 and NWH * WIN == SH

    ND = D // 128        # d-tiles
    NHK = HK // 128      # head-pair tiles (2 heads each)
    NST = S // 128       # s-tiles (full seq)
    NQT = SH // 128      # q-tiles (half seq)
    NSS = WIN // 128     # s-subtiles per window

    f32 = mybir.dt.float32
    bf16 = mybir.dt.bfloat16
    Exp = mybir.ActivationFunctionType.Exp
    Square = mybir.ActivationFunctionType.Square
    Sqrt = mybir.ActivationFunctionType.Sqrt
    add_op = mybir.AluOpType.add
    mult_op = mybir.AluOpType.mult
    sub_op = mybir.AluOpType.subtract
    AxX = mybir.AxisListType.X

    nc = bacc.Bacc("TRN2", target_bir_lowering=False, debug=False,
                   num_devices=N_CORES)

    # ---- DRAM parameters -------------------------------------------------
    xT_d = nc.dram_tensor("xT", [D, S], bf16, kind="ExternalInput")
    xq_d = nc.dram_tensor("xq", [SH, D], f32, kind="ExternalInput")
    w_d = {}
    for nm in ("wq_g", "wk_g", "wv_g", "wq_l", "wk_l", "wv_l"):
        w_d[nm] = nc.dram_tensor(nm, [D, HK], bf16, kind="ExternalInput")
    wo_g_d = nc.dram_tensor("wo_g", [HK, D], bf16, kind="ExternalInput")
    wo_l_d = nc.dram_tensor("wo_l", [HK, D], bf16, kind="ExternalInput")
    bcol_d = {}
    for nm in ("bq_g", "bk_g", "bq_l", "bk_l"):
        bcol_d[nm] = nc.dram_tensor(nm, [NHK, 128], f32, kind="ExternalInput")
    bv_g_d = nc.dram_tensor("bv_g", [1, HK], bf16, kind="ExternalInput")
    bv_l_d = nc.dram_tensor("bv_l", [1, HK], bf16, kind="ExternalInput")
    bo_d = nc.dram_tensor("bo", [1, D], bf16, kind="ExternalInput")
    gamma_d = nc.dram_tensor("gamma", [1, D], f32, kind="ExternalInput")
    beta_d = nc.dram_tensor("beta", [1, D], f32, kind="ExternalInput")
    out_d = nc.dram_tensor("out", [SH, D], f32, kind="ExternalOutput")

    # DRAM scratch for normalized o^T (bf16), per set; frees SBUF across phases
    oscr = {"g": nc.dram_tensor("oscr_g", [HK, SH], bf16),
            "l": nc.dram_tensor("oscr_l", [HK, SH], bf16)}

    PS = bass.MemorySpace.PSUM

    with tile.TileContext(nc) as tc, ExitStack() as ctx:
        # ---- small constants (live whole kernel) -------------------------
        cpool = ctx.enter_context(tc.tile_pool(name="consts", bufs=1))
        ones_bf = cpool.tile([1, 128], bf16, tag="ones", name="ones")
        nc.vector.memset(ones_bf[:], 1.0)
        eps_col = cpool.tile([128, 1], f32, tag="eps", name="eps")
        nc.vector.memset(eps_col[:], float(LN_EPS))
        brow_sb = {}
        for nm, d in (("bv_g", bv_g_d), ("bv_l", bv_l_d), ("bo", bo_d)):
            t = cpool.tile([1, d.shape[1]], bf16, tag=nm)
            nc.sync.dma_start(t[:], d[:])
            brow_sb[nm] = t
        bcol_sb = {}
        for nm, d in bcol_d.items():
            cols = []
            for j in range(NHK):
                t = cpool.tile([128, 1], f32, tag=f"{nm}{j}", name=f"{nm}{j}")
                nc.sync.dma_start(t[:], d[j, :].rearrange("(a b) -> a b", b=1))
                cols.append(t)
            bcol_sb[nm] = cols

        # ---- attention (shared for global/local) -------------------------
        def attention(kT, qT, vx, rounds, o_dst, prefix):
            """rounds: list of (segments, start, stop); segment =
            (q_off, q_len, s_col, vx_idx): scores^T for s-tile at kT column
            s_col vs queries [q_off:q_off+q_len], AV into o_ps[:, q_off:...]
            accumulated with start/stop. o_dst: DRAM [HK, SH] scratch."""
            with tc.tile_pool(name=f"{prefix}sc", bufs=2, space=PS) as scp, \
                 tc.tile_pool(name=f"{prefix}op", bufs=1, space=PS) as opp, \
                 tc.tile_pool(name=f"{prefix}ex", bufs=3) as exp_p, \
                 tc.tile_pool(name=f"{prefix}no", bufs=2) as nop:
                # start/stop must be bank-granular: `start` zeroes the whole
                # 2KB PSUM bank, so only the FIRST matmul touching a bank may
                # set it and only the LAST may stop it.
                first_b, last_b = {}, {}
                for ri, (segs, _, _) in enumerate(rounds):
                    for (qo, ql, _sc, _vx) in segs:
                        for co, cl in _chunks(ql, 512):
                            bank = (qo + co) // 512
                            first_b.setdefault(bank, (ri, qo + co))
                            last_b[bank] = (ri, qo + co)

                for hp in range(NHK):
                    o_ps = [opp.tile([65, SH], f32, tag=f"ops{sub}", name=f"ops{sub}")
                            for sub in range(2)]
                    pend = deque()

                    def do_av(item):
                        sub, ex, segs, ri = item
                        h = 2 * hp + sub
                        for (qo, ql, _scol, vxt) in segs:
                            for co, cl in _chunks(ql, 512):
                                col = qo + co
                                bank = col // 512
                                nc.tensor.matmul(
                                    o_ps[sub][:, col:col + cl],
                                    vx[vxt][:, h, :], ex[:, col:col + cl],
                                    start=(first_b[bank] == (ri, col)),
                                    stop=(last_b[bank] == (ri, col)))

                    for ri, (segs, st_, sp_) in enumerate(rounds):
                        for sub in range(2):
                            po = sub * 64
                            sc = scp.tile([128, SH], f32, tag="sc", name="sc")
                            for (qo, ql, scol, _vxt) in segs:
                                for co, cl in _chunks(ql, 512):
                                    nc.tensor.matmul(
                                        sc[:, qo + co:qo + co + cl],
                                        kT[hp][po:po + 64, scol:scol + 128],
                                        qT[hp][po:po + 64, qo + co:qo + co + cl],
                                        start=True, stop=True)
                            ex = exp_p.tile([128, SH], bf16, tag="ex", name="ex")
                            nc.scalar.activation(ex[:], sc[:], Exp, scale=0.125)
                            pend.append((sub, ex, segs, ri))
                            while len(pend) > 2:
                                do_av(pend.popleft())
                    while pend:
                        do_av(pend.popleft())

                    # normalize by rowsum (row 64) and spill to DRAM scratch
                    for sub in range(2):
                        rinv = nop.tile([1, SH], f32, tag=f"ri{sub}", name=f"ri{sub}")
                        nc.vector.reciprocal(rinv[:], o_ps[sub][64:65, :])
                        rb = nop.tile([64, SH], f32, tag=f"rb{sub}", name=f"rb{sub}")
                        nc.gpsimd.partition_broadcast(rb[:], rinv[0:1, :])
                        oh = nop.tile([64, SH], bf16, tag=f"oh{sub}", name=f"oh{sub}")
                        nc.vector.tensor_tensor(
                            oh[:], o_ps[sub][0:64, :], rb[:], mult_op)
                        nc.sync.dma_start(
                            o_dst[hp * 128 + sub * 64:hp * 128 + sub * 64 + 64, :],
                            oh[:])

        g_rounds = [([(0, SH, st * 128, st)], st == 0, st == NST - 1)
                    for st in range(NST)]
        l_rounds = [([(w * WIN, WIN, (w * NSS + ss) * 128, w * NSS + ss)
                      for w in range(NWH)], ss == 0, ss == NSS - 1)
                    for ss in range(NSS)]

        with tc.tile_pool(name="kqvl", bufs=1) as kqvl:
            kT_l = [kqvl.tile([128, SH], bf16, tag=f"ktl{j}", name=f"ktl{j}") for j in range(NHK)]
            qT_l = [kqvl.tile([128, SH], bf16, tag=f"qtl{j}", name=f"qtl{j}") for j in range(NHK)]
            vx_l = [kqvl.tile([128, H, 65], bf16, tag=f"vxl{t}", name=f"vxl{t}")
                    for t in range(SH // 128)]

            with tc.tile_pool(name="kqvg", bufs=1) as kqvg:
                kT_g = [kqvg.tile([128, S], bf16, tag=f"ktg{j}", name=f"ktg{j}")
                        for j in range(NHK)]
                qT_g = [kqvg.tile([128, SH], bf16, tag=f"qtg{j}", name=f"qtg{j}")
                        for j in range(NHK)]
                vx_g = [kqvg.tile([128, H, 65], bf16, tag=f"vxg{t}", name=f"vxg{t}")
                        for t in range(NST)]

                # ========== Phase A: projections =========================
                with tc.tile_pool(name="xin", bufs=1) as xin, \
                     tc.tile_pool(name="wt", bufs=2) as wt, \
                     tc.tile_pool(name="ppsum", bufs=2, space=PS) as ppsum:

                    xT_sb = [xin.tile([128, S], bf16, tag=f"xt{d}", name=f"xt{d}")
                             for d in range(ND)]
                    for d in range(ND):
                        nc.sync.dma_start(xT_sb[d][:],
                                          xT_d[d * 128:(d + 1) * 128, :])

                    def load_w(nm):
                        ts = []
                        for d in range(ND):
                            t = wt.tile([128, HK], bf16, tag=f"wd{d}", name=f"wd{d}")
                            nc.sync.dma_start(
                                t[:], w_d[nm][d * 128:(d + 1) * 128, :])
                            ts.append(t)
                        return ts

                    def proj_kq(w_tiles, s_len, out_tiles, bias_cols):
                        # out[hk, s] = (x @ w)^T + bias ; hk chunks of 128
                        for j in range(NHK):
                            pt = ppsum.tile([128, s_len], f32, tag="pp", name="pp")
                            for d in range(ND):
                                for so, sl in _chunks(s_len, 512):
                                    nc.tensor.matmul(
                                        pt[:, so:so + sl],
                                        w_tiles[d][:, j * 128:(j + 1) * 128],
                                        xT_sb[d][:, so:so + sl],
                                        start=(d == 0), stop=(d == ND - 1))
                            nc.vector.tensor_scalar(
                                out_tiles[j][:], pt[:], bias_cols[j], None,
                                add_op)

                    def proj_v(w_tiles, s_len, out_tiles, bias_row):
                        # out[s, hk] = x @ w + bias ; s tiles of 128
                        for t in range(s_len // 128):
                            pt = ppsum.tile([128, HK], f32, tag="pp", name="pp")
                            for d in range(ND):
                                for ho, hl in _chunks(HK, 512):
                                    nc.tensor.matmul(
                                        pt[:, ho:ho + hl],
                                        xT_sb[d][:, t * 128:(t + 1) * 128],
                                        w_tiles[d][:, ho:ho + hl],
                                        start=(d == 0), stop=False)
                            for ho, hl in _chunks(HK, 512):
                                nc.tensor.matmul(
                                    pt[:, ho:ho + hl], ones_bf[0:1, 0:128],
                                    bias_row[0:1, ho:ho + hl],
                                    start=False, stop=True)
                            ot = out_tiles[t]
                            nc.vector.memset(ot[:, :, 64:65], 1.0)
                            nc.vector.tensor_copy(
                                ot[:, :, 0:64],
                                pt[:].rearrange("p (h k) -> p h k", k=64))

                    wts = load_w("wk_g")
                    proj_kq(wts, S, kT_g, bcol_sb["bk_g"])
                    wts = load_w("wq_g")
                    proj_kq(wts, SH, qT_g, bcol_sb["bq_g"])
                    wts = load_w("wv_g")
                    proj_v(wts, S, vx_g, brow_sb["bv_g"])
                    wts = load_w("wk_l")
                    proj_kq(wts, SH, kT_l, bcol_sb["bk_l"])
                    wts = load_w("wq_l")
                    proj_kq(wts, SH, qT_l, bcol_sb["bq_l"])
                    wts = load_w("wv_l")
                    proj_v(wts, SH, vx_l, brow_sb["bv_l"])

                # ========== Phase B: global attention ====================
                attention(kT_g, qT_g, vx_g, g_rounds, oscr["g"], "g")

            # ========== Phase C: local attention =========================
            attention(kT_l, qT_l, vx_l, l_rounds, oscr["l"], "l")

        # ========== Phase D: output projection + residual + layernorm ====
        with tc.tile_pool(name="wo", bufs=1) as wop, \
             tc.tile_pool(name="opd", bufs=1) as opd, \
             tc.tile_pool(name="ypsum", bufs=2, space=PS) as ypp, \
             tc.tile_pool(name="ln", bufs=2) as lnp:
            gamma_bc = lnp.tile([128, D], f32, tag="gamma", name="gamma", bufs=1)
            nc.sync.dma_start(gamma_bc[:], gamma_d[:].partition_broadcast(128))
            beta_bc = lnp.tile([128, D], f32, tag="beta", name="beta", bufs=1)
            nc.sync.dma_start(beta_bc[:], beta_d[:].partition_broadcast(128))

            wo_sb, o_sb = {}, {}
            for st_, d in (("g", wo_g_d), ("l", wo_l_d)):
                wo_sb[st_] = [wop.tile([128, D], bf16, tag=f"wo{st_}{t}", name=f"wo{st_}{t}")
                              for t in range(NHK)]
                o_sb[st_] = [opd.tile([128, SH], bf16, tag=f"ob{st_}{t}", name=f"ob{st_}{t}")
                             for t in range(NHK)]
                for t in range(NHK):
                    nc.sync.dma_start(wo_sb[st_][t][:],
                                      d[t * 128:(t + 1) * 128, :])
                    nc.sync.dma_start(o_sb[st_][t][:],
                                      oscr[st_][t * 128:(t + 1) * 128, :])
            for qt in range(NQT):
                ps_y = ypp.tile([128, D], f32, tag="py", name="py")
                for do, dl in _chunks(D, 512):
                    first = True
                    for st_ in ("g", "l"):
                        for t in range(NHK):
                            nc.tensor.matmul(
                                ps_y[:, do:do + dl],
                                o_sb[st_][t][:, qt * 128:(qt + 1) * 128],
                                wo_sb[st_][t][:, do:do + dl],
                                start=first, stop=False)
                            first = False
                    nc.tensor.matmul(
                        ps_y[:, do:do + dl], ones_bf[0:1, 0:128],
                        brow_sb["bo"][0:1, do:do + dl], start=False, stop=True)
                xq_t = lnp.tile([128, D], f32, tag="xq", name="xq")
                nc.sync.dma_start(xq_t[:], xq_d[qt * 128:(qt + 1) * 128, :])
                y = lnp.tile([128, D], f32, tag="y", name="y")
                nc.vector.tensor_tensor(y[:], ps_y[:], xq_t[:], add_op)
                ssum = lnp.tile([128, 1], f32, tag="ssum", name="ssum")
                nc.vector.reduce_sum(ssum[:], y[:], axis=AxX)
                sqd = lnp.tile([128, D], bf16, tag="sqd", name="sqd")
                ssq = lnp.tile([128, 1], f32, tag="ssq", name="ssq")
                nc.scalar.activation(sqd[:], y[:], Square, accum_out=ssq[:])
                mu = lnp.tile([128, 1], f32, tag="mu", name="mu")
                nc.vector.tensor_scalar_mul(mu[:], ssum[:], 1.0 / D)
                var = lnp.tile([128, 1], f32, tag="var", name="var")
                nc.vector.tensor_scalar_mul(var[:], ssq[:], 1.0 / D)
                mu2 = lnp.tile([128, 1], f32, tag="mu2", name="mu2")
                nc.vector.tensor_tensor(mu2[:], mu[:], mu[:], mult_op)
                nc.vector.tensor_tensor(var[:], var[:], mu2[:], sub_op)
                sd = lnp.tile([128, 1], f32, tag="sd", name="sd")
                nc.scalar.activation(sd[:], var[:], Sqrt, bias=eps_col[:])
                rstd = lnp.tile([128, 1], f32, tag="rstd", name="rstd")
                nc.vector.reciprocal(rstd[:], sd[:])
                bco = lnp.tile([128, 1], f32, tag="bco", name="bco")
                nc.vector.tensor_tensor(bco[:], mu[:], rstd[:], mult_op)
                nc.vector.tensor_scalar_mul(bco[:], bco[:], -1.0)
                t1 = lnp.tile([128, D], f32, tag="t1", name="t1")
                nc.vector.tensor_scalar(t1[:], y[:], rstd[:], bco[:],
                                        mult_op, add_op)
                t2 = lnp.tile([128, D], f32, tag="t2", name="t2")
                nc.vector.tensor_tensor(t2[:], t1[:], gamma_bc[:], mult_op)
                ot = lnp.tile([128, D], f32, tag="ot", name="ot")
                nc.vector.tensor_tensor(ot[:], t2[:], beta_bc[:], add_op)
                nc.sync.dma_start(out_d[qt * 128:(qt + 1) * 128, :], ot[:])

    nc.compile()
    return nc


def make_in_maps(inputs, cfg=None):
    """Build per-core input maps from the full (unsharded) problem inputs."""
    cfg = dict(cfg or FULL_CFG)
    S, D, H, K = cfg["S"], cfg["D"], cfg["H"], cfg["K"]
    HK = H * K
    SH = S // 2
    NHK = HK // 128

    def np32(a):
        return np.asarray(a, dtype=np.float32)

    shared = {}
    for nm, key in (("wq_g", "gWq"), ("wk_g", "gWk"), ("wv_g", "gWv"),
                    ("wq_l", "lWq"), ("wk_l", "lWk"), ("wv_l", "lWv")):
        shared[nm] = np.ascontiguousarray(
            np32(inputs[key]).reshape(D, HK)).astype(BF16)
    shared["wo_g"] = np.ascontiguousarray(
        np32(inputs["gWo"]).reshape(HK, D)).astype(BF16)
    shared["wo_l"] = np.ascontiguousarray(
        np32(inputs["lWo"]).reshape(HK, D)).astype(BF16)
    for nm, key in (("bq_g", "gbq"), ("bk_g", "gbk"),
                    ("bq_l", "lbq"), ("bk_l", "lbk")):
        shared[nm] = np.ascontiguousarray(np32(inputs[key]).reshape(NHK, 128))
    shared["bv_g"] = np32(inputs["gbv"]).reshape(1, HK).astype(BF16)
    shared["bv_l"] = np32(inputs["lbv"]).reshape(1, HK).astype(BF16)
    shared["bo"] = (np32(inputs["gbo"]) +
                    np32(inputs["lbo"])).reshape(1, D).astype(BF16)
    shared["gamma"] = np32(inputs["gamma"]).reshape(1, D)
    shared["beta"] = np32(inputs["beta"]).reshape(1, D)

    x = np32(inputs["x"])
    in_maps = []
    for c in range(N_CORES):
        b, half = divmod(c, 2)
        xb = x[b]
        # own half first (queries/local), other half second; global attention
        # is invariant to key/value column order
        xperm = np.concatenate([xb[half * SH:(half + 1) * SH],
                                xb[(1 - half) * SH:(2 - half) * SH]], axis=0)
        m = dict(shared)
        m["xT"] = np.ascontiguousarray(xperm.T).astype(BF16)
        m["xq"] = np.ascontiguousarray(xperm[0:SH])
        in_maps.append(m)
    return in_maps


def assemble_out(results, cfg=None):
    cfg = dict(cfg or FULL_CFG)
    S, D = cfg["S"], cfg["D"]
    SH = S // 2
    B = N_CORES // 2
    out = np.empty((B, S, D), np.float32)
    for c in range(N_CORES):
        b, half = divmod(c, 2)
        out[b, half * SH:(half + 1) * SH] = results[c]["out"]
    return out


_NC_CACHE = {}


def kernel(**inputs):
    from concourse.bass_utils import run_bass_kernel_spmd
    if "nc" not in _NC_CACHE:
        _NC_CACHE["nc"] = build_nc()
    nc = _NC_CACHE["nc"]
    in_maps = make_in_maps(inputs)
    res = run_bass_kernel_spmd(nc, in_maps, list(range(N_CORES)))
    return assemble_out(res.results)

